# revision 1
# baseline (speedup 1.0000x reference)
"""Multi-head self-attention (B=2, N=2048, D=1024, H=16) on 8 Trainium2 cores.

Sharding: core c -> batch b = c // 4, head group g = c % 4 (heads 4g..4g+3).
Each core computes q/k/v for its 4 heads, attention in transposed layout
(scores^T = [j, i] so no big transposes are needed), and a partial output
projection restricted to its heads' rows of w_proj.  The host transposes x
once per batch on the way in, and sums the 4 per-core partial projections
(+ residual x) per batch on the way out.

Per-core kernel layout (matmul operands float32r, fp32 elsewhere):
  P1  qkv:   qT/kT per head-pair [128=2*64 dims, 2048 tok], v [tok, 256]
  P2  attn:  per pair, per i-tile(512), per j-chunk(128):
               scores^T row-packed pair (K=64 x2) -> psum [128, 1024]
               exp via one ACT activate (scale=1/8), no max subtraction
               PV col-packed (M=32 x4) + denominator rows in spare col slots
             then reciprocal + PE-broadcast + DVE normalize -> outT [dd, tok]
  P3  proj:  partial^T [1024, 2048] = wp^T-chunks @ outT, DVE evac, DMA out
"""

import numpy as np

import concourse.bass as bass
import concourse.bacc as bacc
import concourse.mybir as mybir
import concourse.tile as tile
from concourse.bass_utils import run_bass_kernel_spmd

B = 2
N = 2048
D = 1024
NH = 16
DH = 64
N_CORES = 8
TP = 4                # head-parallel ways per batch
HPC = NH // TP        # heads per core
HDIM = HPC * DH       # 256 head dims per core
PAIRS = HPC // 2
SCALE = 1.0 / 8.0     # 1/sqrt(DH)

IT = N // 512         # 4 i-tiles
JT = N // 128         # 16 j-chunks
KC = D // 128         # 8 feature chunks

F32 = mybir.dt.float32
F32R = mybir.dt.float32r
AF = mybir.ActivationFunctionType


def build_bass():
    nc = bacc.Bacc("TRN2", target_bir_lowering=False, debug=False)
    xT = nc.declare_dram_parameter("xT", [D, N], F32R, isOutput=False)
    wq = nc.declare_dram_parameter("wq", [D, HDIM], F32R, isOutput=False)
    wk = nc.declare_dram_parameter("wk", [D, HDIM], F32R, isOutput=False)
    wv = nc.declare_dram_parameter("wv", [D, HDIM], F32R, isOutput=False)
    wp = nc.declare_dram_parameter("wp", [HDIM, D], F32R, isOutput=False)
    ones4_d = nc.declare_dram_parameter("ones4_c", [128, 4], F32R, isOutput=False)
    selAB_d = nc.declare_dram_parameter("selAB_c", [128, 128], F32R, isOutput=False)
    zeros_d = nc.declare_dram_parameter("zeros_c", [128, 512], F32R, isOutput=False)
    pT0 = nc.declare_dram_parameter("pT0", [D, N], F32, isOutput=True)
    pT1 = nc.declare_dram_parameter("pT1", [D, N], F32, isOutput=True)

    with tile.TileContext(nc) as tc:
        with (
            tc.tile_pool(name="big", bufs=1) as big,
            tc.tile_pool(name="exps", bufs=3) as exps,
            tc.tile_pool(name="evac", bufs=4) as evac,
            tc.tile_pool(name="psum", bufs=1, space="PSUM") as psum,
        ):
            # ---- constants (DMA'd: memset can't produce f32r) ----
            # selAB[k, m] = 1 iff (k==0, m<64) or (k==64, m>=64): broadcast matmul
            selAB = big.tile([128, 128], F32R, tag="selAB")
            nc.sync.dma_start(out=selAB, in_=selAB_d[:, :])
            # recip_pad rows 1..63 and 65..127 must stay zero; rows 0/64 are
            # rewritten with the softmax reciprocals each i-tile.
            recip_pad = big.tile([128, 512], F32R, tag="recip_pad")
            nc.sync.dma_start(out=recip_pad, in_=zeros_d[:, :])

            # ---- P1: load inputs (per-chunk tiles keep sync-wait fan-in low) ----
            xts, wqs, wks, wvs = [], [], [], []
            for k in range(KC):
                for lst, nm, src_t, w in (
                    (xts, "xt", xT, N), (wqs, "wq", wq, HDIM),
                    (wks, "wk", wk, HDIM), (wvs, "wv", wv, HDIM),
                ):
                    t = big.tile([128, w], F32R, tag=f"{nm}{k}")
                    eng = nc.sync if (k % 2 == 0) else nc.scalar
                    eng.dma_start(out=t, in_=src_t[k * 128:(k + 1) * 128, :])
                    lst.append(t)
            wps = []
            for p in range(PAIRS):
                t = big.tile([128, D], F32R, tag=f"wp{p}")
                nc.sync.dma_start(out=t, in_=wp[p * 128:(p + 1) * 128, :])
                wps.append(t)

            # ---- P1: qT/kT pair 0 now; pair 1 + v interleaved into P2 ----
            qT = big.tile([128, PAIRS, N], F32R, tag="qT")
            kT = big.tile([128, PAIRS, N], F32R, tag="kT")

            def emit_qk_tile(p, w_s, dst, nt):
                ps = psum.tile([128, 512], F32, tag="mm", bufs=2)
                for k in range(KC):
                    nc.tensor.matmul(
                        ps,
                        lhsT=w_s[k][:, p * 128:(p + 1) * 128],
                        rhs=xts[k][:, nt * 512:(nt + 1) * 512],
                        start=(k == 0),
                        stop=(k == KC - 1),
                    )
                nc.vector.tensor_copy(dst[:, p, nt * 512:(nt + 1) * 512], ps)

            for w_s, dst in ((wqs, qT), (wks, kT)):
                for nt in range(IT):
                    emit_qk_tile(0, w_s, dst, nt)

            # v_aug: per head [v_h | ones] = 65 cols; pair p head h at
            # offset (2p+h)*65.  PV matmul lhsT [128, 65] then yields the
            # softmax denominator as output row 64 for free.
            v_s = big.tile([128, JT, 4 * 65], F32R, tag="v")

            def emit_v_chunk(t):
                nc.sync.dma_start(
                    out=v_s[:, t, :].rearrange("p (h c) -> p h c", c=65)[:, :, 64:65],
                    in_=ones4_d[:, :].unsqueeze(2),
                )
                ps = psum.tile([128, HDIM], F32, tag="mm", bufs=2)
                for k in range(KC):
                    nc.tensor.matmul(
                        ps,
                        lhsT=xts[k][:, t * 128:(t + 1) * 128],
                        rhs=wvs[k],
                        start=(k == 0),
                        stop=(k == KC - 1),
                    )
                nc.vector.tensor_copy(
                    v_s[:, t, :].rearrange("p (h c) -> p h c", c=65)[:, :, 0:64],
                    ps.rearrange("p (h c) -> p h c", c=64),
                )

            # ---- P2: attention (transposed flow) ----
            warm = evac.tile([1, 1], F32, tag="warm")
            nc.scalar.activation(warm, selAB[0:1, 0:1], AF.Exp)
            outTn = big.tile([128, PAIRS, N], F32R, tag="outTn")

            def emit_scores(p, it, jt):
                sc = psum.tile([128, 1024], F32, tag="sc", bufs=2)
                for h in range(2):
                    nc.tensor.matmul(
                        sc[:, h * 512:(h + 1) * 512],
                        lhsT=kT[h * 64:(h + 1) * 64, p, jt * 128:(jt + 1) * 128],
                        rhs=qT[h * 64:(h + 1) * 64, p, it * 512:(it + 1) * 512],
                        start=True,
                        stop=True,
                    )
                return sc

            def emit_proj_tile(p, dst, ot, tt):
                pj = psum.tile([128, 512], F32, tag="mm", bufs=2)
                nc.tensor.matmul(
                    pj,
                    lhsT=wps[p][:, ot * 128:(ot + 1) * 128],
                    rhs=outTn[:, p, tt * 512:(tt + 1) * 512],
                    start=True,
                    stop=True,
                )
                o_sb = evac.tile([128, 512], F32, tag="osb")
                nc.vector.tensor_copy(o_sb, pj)
                nc.sync.dma_start(
                    out=dst[ot * 128:(ot + 1) * 128, tt * 512:(tt + 1) * 512],
                    in_=o_sb,
                )

            qk1 = [(w_s, dst, nt) for w_s, dst in ((wqs, qT), (wks, kT))
                   for nt in range(IT)]
            proj_q = []  # (p, ot, tt) tiles whose outTn slice is complete
            for p in range(PAIRS):
                for it in range(IT):
                    pvA = psum.tile([128, 512], F32, tag="pvA", bufs=1)
                    pvB = psum.tile([128, 512], F32, tag="pvB", bufs=1)
                    sc_next = emit_scores(p, it, 0)
                    for jt in range(JT):
                        sc = sc_next
                        if jt + 1 < JT:
                            sc_next = emit_scores(p, it, jt + 1)
                        if p == 0 and it == 0:
                            emit_v_chunk(jt)  # ready before this jt's PV
                        elif p == 0 and it in (1, 2) and jt % 4 == 0:
                            w_s, dst, nt = qk1.pop(0)  # qk pair 1, 8 tiles
                            emit_qk_tile(1, w_s, dst, nt)
                        elif jt % 2 == 1 and proj_q:
                            pp, ot, tt = proj_q.pop(0)  # stream proj tiles
                            emit_proj_tile(pp, pT0 if pp == 0 else pT1, ot, tt)
                        e = exps.tile([128, 1024], F32R, tag="e")
                        nc.scalar.activation(e, sc, AF.Exp, scale=SCALE)
                        st, sp = (jt == 0), (jt == JT - 1)
                        for h, pvx in ((0, pvA), (1, pvB)):
                            off = (2 * p + h) * 65
                            nc.tensor.matmul(
                                pvx[0:65, :],
                                lhsT=v_s[:, jt, off:off + 65],
                                rhs=e[:, h * 512:(h + 1) * 512],
                                start=st,
                                stop=sp,
                            )
                    # normalize: outTn[:, p, i-tile] = pv * (1/den) bcast over rows
                    with nc.allow_low_precision(reason="f32r softmax denom"):
                        nc.vector.reciprocal(recip_pad[0:1, :], pvA[64:65, :])
                        nc.vector.reciprocal(recip_pad[64:65, :], pvB[64:65, :])
                    bc = psum.tile([128, 1024], F32, tag="sc", bufs=2)
                    nc.tensor.matmul(
                        bc[:, 0:512], lhsT=selAB, rhs=recip_pad,
                        start=True, stop=True,
                    )
                    bc_sb = evac.tile([128, 512], F32, tag="bc")
                    nc.vector.tensor_copy(bc_sb, bc[:, 0:512])
                    nc.vector.tensor_mul(
                        outTn[0:64, p, it * 512:(it + 1) * 512],
                        pvA[0:64, :], bc_sb[0:64, :],
                    )
                    nc.vector.tensor_mul(
                        outTn[64:128, p, it * 512:(it + 1) * 512],
                        pvB[0:64, :], bc_sb[64:128, :],
                    )
                    proj_q.extend((p, ot, it) for ot in range(D // 128))

            # ---- P3: drain remaining proj tiles ----
            for pp, ot, tt in proj_q:
                emit_proj_tile(pp, pT0 if pp == 0 else pT1, ot, tt)
    return nc


_NC = None


def _get_nc():
    global _NC
    if _NC is None:
        _NC = build_bass()
        _NC.finalize()
    return _NC


_ONES4 = np.ones((128, 4), np.float32)
_SELAB = np.zeros((128, 128), np.float32)
_SELAB[0, 0:64] = 1.0
_SELAB[64, 64:128] = 1.0
_ZEROS = np.zeros((128, 512), np.float32)


def make_in_maps(x, w_qkv, w_proj):
    x = np.ascontiguousarray(np.asarray(x, np.float32))
    w_qkv = np.ascontiguousarray(np.asarray(w_qkv, np.float32))
    w_proj = np.ascontiguousarray(np.asarray(w_proj, np.float32))
    xTs = [np.ascontiguousarray(x[b].T) for b in range(B)]
    in_maps = []
    for c in range(N_CORES):
        b, g = divmod(c, TP)
        h0 = g * HDIM
        in_maps.append({
            "xT": xTs[b],
            "wq": np.ascontiguousarray(w_qkv[:, h0:h0 + HDIM]),
            "wk": np.ascontiguousarray(w_qkv[:, D + h0:D + h0 + HDIM]),
            "wv": np.ascontiguousarray(w_qkv[:, 2 * D + h0:2 * D + h0 + HDIM]),
            "wp": np.ascontiguousarray(w_proj[h0:h0 + HDIM, :]),
            "ones4_c": _ONES4,
            "selAB_c": _SELAB,
            "zeros_c": _ZEROS,
        })
    return in_maps


def combine_outputs(x, results):
    x = np.asarray(x, np.float32)
    out = np.empty((B, N, D), np.float32)
    for b in range(B):
        acc = x[b].astype(np.float64)
        for g in range(TP):
            acc += results[b * TP + g]["pT0"].T
            acc += results[b * TP + g]["pT1"].T
        out[b] = acc.astype(np.float32)
    return out


def kernel(x, w_qkv, w_proj):
    nc = _get_nc()
    in_maps = make_in_maps(x, w_qkv, w_proj)
    res = run_bass_kernel_spmd(nc, in_maps, list(range(N_CORES))).results
    return combine_outputs(x, res)

